# revision 11
# baseline (speedup 1.0000x reference)
"""Trainium2 Bass kernel for RandomSparseNewMlp.

Math (reference):
    attn = (einsum('ds,td->st', fc1_w, fc2_w) + fc2_b) * sparse_mask   # [1024, 1024]
    out  = gelu_erf(einsum('bds,st->bdt', x, attn))                    # [64, 768, 1024]

Strategy (8 cores, SPMD, no collectives):
  - Data-parallel shard x over batch: core c handles rows [c*6144, (c+1)*6144)
    of the flattened [49152, 1024] x.
  - Every core redundantly computes the full [1024,1024] attn matrix
    (bf16 weights, fp32 PSUM accumulation).  Bias is folded into the
    contraction as an extra K-row (ones row in fc1, bias row in fc2^T),
    K padded 4096 -> 4224 = 33*128.
  - Main GEMM runs in float32r (tf32-like, full PE rate at N=512) on
    host-pre-transposed x (xT layout [1024, rows]) so the contraction dim
    lands on SBUF partitions with clean DMA.
  - GELU (erf-exact) fused into the PSUM->SBUF eviction on ScalarE.
"""

import numpy as np
import ml_dtypes
from contextlib import ExitStack

import concourse.bass as bass  # noqa: F401  (engine registration side effects)
import concourse.mybir as mybir
import concourse.tile as tile
from concourse import bacc
from concourse import bass_utils

P = 128
B, D = 64, 768
IN_F, HID_F, OUT_F = 1024, 4096, 1024
N_CORES = 8
ROWS = B * D                    # 49152
ROWS_PC = ROWS // N_CORES       # 6144
KH = HID_F + P                  # 4224 = 33*128 (hidden + bias/ones row, padded)
KD = KH // P                    # 33
S_TILES = IN_F // P             # 8
K_CHUNKS = IN_F // P            # 8
RT = ROWS_PC // P               # 48
NB = 512                        # matmul moving free dim / PSUM bank

F32 = mybir.dt.float32
F32R = mybir.dt.float32r
BF16 = mybir.dt.bfloat16


def _trace_kernel(tc, out, fc1e, fc2te, mask, xt):
    nc = tc.nc
    gelu = mybir.ActivationFunctionType.Gelu

    fc1e_r = fc1e.rearrange("(k p) s -> p k s", p=P)    # [128, 33, 1024]
    fc2te_r = fc2te.rearrange("(k p) t -> p k t", p=P)  # [128, 33, 1024]
    mask_r = mask.rearrange("(j p) t -> p j t", p=P)    # [128, 8, 1024]
    xt_r = xt.rearrange("(k p) r -> p k r", p=P)        # [128, 8, 6144]

    with ExitStack() as ctx:
        consts = ctx.enter_context(tc.tile_pool(name="consts", bufs=1))

        # Resident fc2^T strips: [128, 33, 1024] bf16 (66 KB/partition)
        fc2_sb = consts.tile([P, KD, OUT_F], BF16)
        for kd in range(KD):
            nc.sync.dma_start(fc2_sb[:, kd, :], fc2te_r[:, kd, :])

        # attn (masked, bias-folded) lives here for the main GEMM.
        # float32r so the DVE mask-multiply emits the f32r rounding the
        # walrus verifier requires of float32r-matmul producers.
        attn_sb = consts.tile([P, S_TILES, OUT_F], F32R)
        mask_sb = consts.tile([P, S_TILES, OUT_F], F32)
        for j in range(S_TILES):
            nc.sync.dma_start(mask_sb[:, j, :], mask_r[:, j, :])

        # ---- Phase 1: attn = (fc1^T @ fc2^T + b) * mask, all 8 s-tiles ----
        # PSUM has 8 banks; process s-tiles in 2 groups of 4 (x 2 t-blocks).
        wpool = ctx.enter_context(tc.tile_pool(name="wpool", bufs=4))
        with tc.tile_pool(name="attn_psum", bufs=8, space="PSUM") as appool:
            for g in range(2):
                psums = {}
                for si in range(4):
                    for tb in range(2):
                        psums[(si, tb)] = appool.tile([P, NB], F32, name="ap")
                for kd in range(KD):
                    f1 = wpool.tile([P, 4, P], BF16, name="f1")
                    nc.sync.dma_start(
                        f1, fc1e_r[:, kd, g * 512:(g + 1) * 512].rearrange(
                            "p (si c) -> p si c", si=4
                        )
                    )
                    for si in range(4):
                        for tb in range(2):
                            nc.tensor.matmul(
                                psums[(si, tb)],
                                f1[:, si, :],
                                fc2_sb[:, kd, tb * NB:(tb + 1) * NB],
                                start=(kd == 0),
                                stop=(kd == KD - 1),
                            )
                for si in range(4):
                    j = g * 4 + si
                    for tb in range(2):
                        nc.vector.tensor_mul(
                            attn_sb[:, j, tb * NB:(tb + 1) * NB],
                            psums[(si, tb)],
                            mask_sb[:, j, tb * NB:(tb + 1) * NB],
                        )

        # ---- Phase 2: out = gelu(x @ attn), rows tiled by 128 ----
        xpool = ctx.enter_context(tc.tile_pool(name="xpool", bufs=3))
        opool = ctx.enter_context(tc.tile_pool(name="opool", bufs=3))
        mpool = ctx.enter_context(tc.tile_pool(name="main_psum", bufs=6, space="PSUM"))
        for rt in range(RT):
            xs = xpool.tile([P, K_CHUNKS, P], F32R, name="xs")
            nc.sync.dma_start(xs, xt_r[:, :, rt * P:(rt + 1) * P])
            pa = mpool.tile([P, NB], F32, name="mp")
            pb = mpool.tile([P, NB], F32, name="mp")
            for k in range(K_CHUNKS):
                nc.tensor.matmul(
                    pa,
                    xs[:, k, :],
                    attn_sb[:, k, 0:NB],
                    start=(k == 0),
                    stop=(k == K_CHUNKS - 1),
                )
            for k in range(K_CHUNKS):
                nc.tensor.matmul(
                    pb,
                    xs[:, k, :],
                    attn_sb[:, k, NB:OUT_F],
                    start=(k == 0),
                    stop=(k == K_CHUNKS - 1),
                )
            ot = opool.tile([P, OUT_F], F32, name="ot")
            nc.scalar.activation(ot[:, 0:NB], pa, gelu)
            nc.scalar.activation(ot[:, NB:OUT_F], pb, gelu)
            nc.sync.dma_start(out[rt * P:(rt + 1) * P, :], ot)


_NC_CACHE = None
LAST_RESULTS = None


def _build():
    global _NC_CACHE
    if _NC_CACHE is not None:
        return _NC_CACHE
    nc = bacc.Bacc("TRN2", target_bir_lowering=False, debug=False,
                   num_devices=N_CORES)
    fc1e = nc.dram_tensor("fc1e", [KH, IN_F], BF16, kind="ExternalInput").ap()
    fc2te = nc.dram_tensor("fc2te", [KH, OUT_F], BF16, kind="ExternalInput").ap()
    mask = nc.dram_tensor("mask", [IN_F, OUT_F], F32, kind="ExternalInput").ap()
    xt = nc.dram_tensor("xt", [IN_F, ROWS_PC], F32R, kind="ExternalInput").ap()
    out = nc.dram_tensor("out", [ROWS_PC, OUT_F], F32, kind="ExternalOutput").ap()
    with tile.TileContext(nc) as tc:
        _trace_kernel(tc, out, fc1e, fc2te, mask, xt)
    nc.compile()
    _NC_CACHE = nc
    return nc


def _run(nc, in_maps, **kwargs):
    return bass_utils.run_bass_kernel_spmd(
        nc, in_maps, core_ids=list(range(N_CORES)), **kwargs
    )


def _prep_inputs(x, fc1_w, fc2_w, fc2_b, sparse_mask):
    bf = ml_dtypes.bfloat16
    fc1e = np.concatenate(
        [
            np.asarray(fc1_w, np.float32),
            np.ones((1, IN_F), np.float32),
            np.zeros((P - 1, IN_F), np.float32),
        ],
        axis=0,
    ).astype(bf)
    fc2te = np.concatenate(
        [
            np.asarray(fc2_w, np.float32).T,
            np.asarray(fc2_b, np.float32)[None, :],
            np.zeros((P - 1, OUT_F), np.float32),
        ],
        axis=0,
    ).astype(bf)
    mask = np.ascontiguousarray(np.asarray(sparse_mask, np.float32))
    x_flat = np.asarray(x, np.float32).reshape(ROWS, IN_F)
    in_maps = []
    for c in range(N_CORES):
        xt_c = np.ascontiguousarray(
            x_flat[c * ROWS_PC:(c + 1) * ROWS_PC].T
        )
        in_maps.append(
            {"fc1e": fc1e, "fc2te": fc2te, "mask": mask, "xt": xt_c}
        )
    return in_maps


def kernel(x, fc1_w, fc2_w, fc2_b, sparse_mask, **run_kwargs):
    global LAST_RESULTS
    nc = _build()
    in_maps = _prep_inputs(x, fc1_w, fc2_w, fc2_b, sparse_mask)
    res = _run(nc, in_maps, **run_kwargs)
    LAST_RESULTS = res
    out = np.concatenate(
        [res.results[c]["out"] for c in range(N_CORES)], axis=0
    )
    return out.reshape(B, D, OUT_F)


# revision 12
# speedup vs baseline: 1.3591x; 1.3591x over previous
"""Trainium2 Bass kernel for RandomSparseNewMlp.

Math (reference):
    attn = (einsum('ds,td->st', fc1_w, fc2_w) + fc2_b) * sparse_mask   # [1024, 1024]
    out  = gelu_erf(einsum('bds,st->bdt', x, attn))                    # [64, 768, 1024]

Strategy (8 cores, SPMD, no collectives):
  - Data-parallel shard x over batch: core c handles rows [c*6144, (c+1)*6144)
    of the flattened [49152, 1024] x.
  - Every core redundantly computes the full [1024,1024] attn matrix.
    Bias is folded into the contraction as an extra K-row (ones row in
    fc1, bias row in fc2^T), K padded 4096 -> 4224 = 33*128.
  - All matmul operands are fp16: full PE rate (1 cycle/row), 2-byte
    weight loads (LDWEIGHTS < moving-stream time, so it hides), ~5e-4
    element precision, and half the HBM traffic of fp32.  PSUM
    accumulation stays fp32.
  - x is host-pre-transposed (xT layout [1024, rows]) so the contraction
    dim lands on SBUF partitions with clean, contiguous DMA.
  - GELU (erf-exact) fused into the PSUM->SBUF eviction on ScalarE.
"""

import numpy as np
from contextlib import ExitStack

import concourse.bass as bass  # noqa: F401  (engine registration side effects)
import concourse.mybir as mybir
import concourse.tile as tile
from concourse import bacc
from concourse import bass_utils

P = 128
B, D = 64, 768
IN_F, HID_F, OUT_F = 1024, 4096, 1024
N_CORES = 8
ROWS = B * D                    # 49152
ROWS_PC = ROWS // N_CORES       # 6144
KH = HID_F + P                  # 4224 = 33*128 (hidden + bias/ones row, padded)
KD = KH // P                    # 33
S_TILES = IN_F // P             # 8
K_CHUNKS = IN_F // P            # 8
RT = ROWS_PC // P               # 48
NB = 512                        # matmul moving free dim / PSUM bank

F32 = mybir.dt.float32
F16 = mybir.dt.float16


def _trace_kernel(tc, out, fc1e, fc2te, mask, xt):
    nc = tc.nc
    gelu = mybir.ActivationFunctionType.Gelu

    fc1e_r = fc1e.rearrange("(k p) s -> p k s", p=P)    # [128, 33, 1024]
    fc2te_r = fc2te.rearrange("(k p) t -> p k t", p=P)  # [128, 33, 1024]
    mask_r = mask.rearrange("(j p) t -> p j t", p=P)    # [128, 8, 1024]
    xt_r = xt.rearrange("(k p) r -> p k r", p=P)        # [128, 8, 6144]

    with ExitStack() as ctx:
        consts = ctx.enter_context(tc.tile_pool(name="consts", bufs=1))

        # Resident fc2^T strips: [128, 33, 1024] fp16 (66 KB/partition).
        # Loaded strip-by-strip inside the first k-loop so matmuls can
        # chase the DMAs instead of waiting for the full tensor.
        fc2_sb = consts.tile([P, KD, OUT_F], F16)
        # attn (masked, bias-folded), fp16 operand of the main GEMM
        attn_sb = consts.tile([P, S_TILES, OUT_F], F16)
        mask_sb = consts.tile([P, S_TILES, OUT_F], F32)

        # ---- Phase 1: attn = (fc1^T @ fc2^T + b) * mask, all 8 s-tiles ----
        # PSUM has 8 banks; process s-tiles in 2 groups of 4 (x 2 t-blocks).
        wpool = ctx.enter_context(tc.tile_pool(name="wpool", bufs=6))
        xpool = ctx.enter_context(tc.tile_pool(name="xpool", bufs=6))
        with tc.tile_pool(name="attn_psum", bufs=8, space="PSUM") as appool:
            for g in range(2):
                psums = {}
                for si in range(4):
                    for tb in range(2):
                        psums[(si, tb)] = appool.tile([P, NB], F32, name="ap")
                for kd in range(KD):
                    if g == 0:
                        nc.sync.dma_start(fc2_sb[:, kd, :], fc2te_r[:, kd, :])
                    f1 = wpool.tile([P, 4, P], F16, name="f1")
                    nc.sync.dma_start(
                        f1, fc1e_r[:, kd, g * 512:(g + 1) * 512].rearrange(
                            "p (si c) -> p si c", si=4
                        )
                    )
                    for si in range(4):
                        for tb in range(2):
                            nc.tensor.matmul(
                                psums[(si, tb)],
                                f1[:, si, :],
                                fc2_sb[:, kd, tb * NB:(tb + 1) * NB],
                                start=(kd == 0),
                                stop=(kd == KD - 1),
                            )
                for j in range(g * 4, g * 4 + 4):
                    nc.sync.dma_start(mask_sb[:, j, :], mask_r[:, j, :])
                for si in range(4):
                    j = g * 4 + si
                    for tb in range(2):
                        nc.vector.tensor_mul(
                            attn_sb[:, j, tb * NB:(tb + 1) * NB],
                            psums[(si, tb)],
                            mask_sb[:, j, tb * NB:(tb + 1) * NB],
                        )

        # ---- Phase 2: out = gelu(x @ attn), rows tiled by 128 ----
        opool = ctx.enter_context(tc.tile_pool(name="opool", bufs=3))
        mpool = ctx.enter_context(tc.tile_pool(name="main_psum", bufs=6, space="PSUM"))
        for rt in range(RT):
            xs = xpool.tile([P, K_CHUNKS, P], F16, name="xs")
            nc.sync.dma_start(xs, xt_r[:, :, rt * P:(rt + 1) * P])
            pa = mpool.tile([P, NB], F32, name="mp")
            pb = mpool.tile([P, NB], F32, name="mp")
            for k in range(K_CHUNKS):
                nc.tensor.matmul(
                    pa,
                    xs[:, k, :],
                    attn_sb[:, k, 0:NB],
                    start=(k == 0),
                    stop=(k == K_CHUNKS - 1),
                )
            for k in range(K_CHUNKS):
                nc.tensor.matmul(
                    pb,
                    xs[:, k, :],
                    attn_sb[:, k, NB:OUT_F],
                    start=(k == 0),
                    stop=(k == K_CHUNKS - 1),
                )
            ot = opool.tile([P, OUT_F], F32, name="ot")
            nc.scalar.activation(ot[:, 0:NB], pa, gelu)
            nc.scalar.activation(ot[:, NB:OUT_F], pb, gelu)
            nc.sync.dma_start(out[rt * P:(rt + 1) * P, :], ot)


_NC_CACHE = None
LAST_RESULTS = None


def _build():
    global _NC_CACHE
    if _NC_CACHE is not None:
        return _NC_CACHE
    nc = bacc.Bacc("TRN2", target_bir_lowering=False, debug=False,
                   num_devices=N_CORES)
    fc1e = nc.dram_tensor("fc1e", [KH, IN_F], F16, kind="ExternalInput").ap()
    fc2te = nc.dram_tensor("fc2te", [KH, OUT_F], F16, kind="ExternalInput").ap()
    mask = nc.dram_tensor("mask", [IN_F, OUT_F], F32, kind="ExternalInput").ap()
    xt = nc.dram_tensor("xt", [IN_F, ROWS_PC], F16, kind="ExternalInput").ap()
    out = nc.dram_tensor("out", [ROWS_PC, OUT_F], F32, kind="ExternalOutput").ap()
    with tile.TileContext(nc) as tc:
        _trace_kernel(tc, out, fc1e, fc2te, mask, xt)
    nc.compile()
    _NC_CACHE = nc
    return nc


def _run(nc, in_maps, **kwargs):
    return bass_utils.run_bass_kernel_spmd(
        nc, in_maps, core_ids=list(range(N_CORES)), **kwargs
    )


def _prep_inputs(x, fc1_w, fc2_w, fc2_b, sparse_mask):
    fc1e = np.concatenate(
        [
            np.asarray(fc1_w, np.float32),
            np.ones((1, IN_F), np.float32),
            np.zeros((P - 1, IN_F), np.float32),
        ],
        axis=0,
    ).astype(np.float16)
    fc2te = np.concatenate(
        [
            np.asarray(fc2_w, np.float32).T,
            np.asarray(fc2_b, np.float32)[None, :],
            np.zeros((P - 1, OUT_F), np.float32),
        ],
        axis=0,
    ).astype(np.float16)
    mask = np.ascontiguousarray(np.asarray(sparse_mask, np.float32))
    x_flat = np.asarray(x, np.float32).reshape(ROWS, IN_F)
    in_maps = []
    for c in range(N_CORES):
        xt_c = np.ascontiguousarray(
            x_flat[c * ROWS_PC:(c + 1) * ROWS_PC].T.astype(np.float16)
        )
        in_maps.append(
            {"fc1e": fc1e, "fc2te": fc2te, "mask": mask, "xt": xt_c}
        )
    return in_maps


def kernel(x, fc1_w, fc2_w, fc2_b, sparse_mask, **run_kwargs):
    global LAST_RESULTS
    nc = _build()
    in_maps = _prep_inputs(x, fc1_w, fc2_w, fc2_b, sparse_mask)
    res = _run(nc, in_maps, **run_kwargs)
    LAST_RESULTS = res
    out = np.concatenate(
        [res.results[c]["out"] for c in range(N_CORES)], axis=0
    )
    return out.reshape(B, D, OUT_F)


# revision 13
# speedup vs baseline: 1.6345x; 1.2027x over previous
"""Trainium2 Bass kernel for RandomSparseNewMlp.

Math (reference):
    attn = (einsum('ds,td->st', fc1_w, fc2_w) + fc2_b) * sparse_mask   # [1024, 1024]
    out  = gelu_erf(einsum('bds,st->bdt', x, attn))                    # [64, 768, 1024]

Strategy (8 cores, SPMD, two NEFF dispatches, no collectives):
  NEFF A ("attn"): the [1024,1024] attn matrix is 2D-sharded over the 8
    cores (4-way along s, 2-way along t) — each core computes one
    [256, 512] slice from its fc1/fc2^T column slices, applies bias
    (folded into the contraction as an extra K-row: ones row in fc1,
    bias row in fc2^T, K padded 4096 -> 4224 = 33*128) and the sparse
    mask, and returns the masked fp16 slice.  The host concatenates the
    8 slices (pure layout, no arithmetic).
  NEFF B ("mlp"): data-parallel shard of x over batch; core c handles
    rows [c*6144, (c+1)*6144) of the flattened [49152, 1024] x, computes
    gelu(x @ attn) with the gathered attn as a replicated input.

  All matmul operands are fp16: full PE rate (1 cycle/row), 2-byte
  weight loads (LDWEIGHTS hides under the moving-operand stream), ~5e-4
  element precision, half the HBM traffic of fp32.  PSUM accumulation
  is fp32.  x is host-pre-transposed (xT layout [1024, rows]) so the
  contraction dim lands on SBUF partitions with clean contiguous DMA.
  GELU (erf-exact) is fused into the PSUM->SBUF eviction on ScalarE.
"""

import numpy as np
from contextlib import ExitStack

import concourse.bass as bass  # noqa: F401  (engine registration side effects)
import concourse.mybir as mybir
import concourse.tile as tile
from concourse import bacc
from concourse import bass_utils

P = 128
B, D = 64, 768
IN_F, HID_F, OUT_F = 1024, 4096, 1024
N_CORES = 8
ROWS = B * D                    # 49152
ROWS_PC = ROWS // N_CORES       # 6144
KH = HID_F + P                  # 4224 = 33*128 (hidden + bias/ones row, padded)
KD = KH // P                    # 33
S_TILES = IN_F // P             # 8
K_CHUNKS = IN_F // P            # 8
RT = ROWS_PC // P               # 48
NB = 512                        # matmul moving free dim / PSUM bank
S_SH, T_SH = 4, 2               # attn sharding grid: 4 along s, 2 along t
S_SL = IN_F // S_SH             # 256 rows of attn per core
T_SL = OUT_F // T_SH            # 512 cols of attn per core

F32 = mybir.dt.float32
F16 = mybir.dt.float16


def _trace_attn_kernel(tc, aslice, fc1s, fc2ts, masks):
    """Per-core attn slice: aslice[256,512] = (fc1s^T @ fc2ts) * masks.

    fc1s  [4224, 256]  fp16 : fc1 (K-extended) columns for this core's s-rows
    fc2ts [4224, 512]  fp16 : fc2^T (K-extended) columns for this core's t-cols
    masks [256, 512]   f32  : sparse-mask slice
    """
    nc = tc.nc
    fc1_r = fc1s.rearrange("(k p) s -> p k s", p=P)     # [128, 33, 256]
    fc2_r = fc2ts.rearrange("(k p) t -> p k t", p=P)    # [128, 33, 512]
    mask_r = masks.rearrange("(j p) t -> p j t", p=P)   # [128, 2, 512]

    with ExitStack() as ctx:
        wpool = ctx.enter_context(tc.tile_pool(name="wpool", bufs=6))
        spool = ctx.enter_context(tc.tile_pool(name="spool", bufs=1))
        ppool = ctx.enter_context(tc.tile_pool(name="ppool", bufs=2, space="PSUM"))
        mask_sb = spool.tile([P, 2, T_SL], F32)
        out_sb = spool.tile([P, 2, T_SL], F16)
        psums = [ppool.tile([P, NB], F32, name="ap") for _ in range(2)]
        for kd in range(KD):
            f2 = wpool.tile([P, T_SL], F16, name="f2")
            nc.sync.dma_start(f2, fc2_r[:, kd, :])
            f1 = wpool.tile([P, 2, P], F16, name="f1")
            nc.sync.dma_start(
                f1, fc1_r[:, kd, :].rearrange("p (si c) -> p si c", si=2)
            )
            for si in range(2):
                nc.tensor.matmul(
                    psums[si],
                    f1[:, si, :],
                    f2,
                    start=(kd == 0),
                    stop=(kd == KD - 1),
                )
        for j in range(2):
            nc.sync.dma_start(mask_sb[:, j, :], mask_r[:, j, :])
        for si in range(2):
            nc.vector.tensor_mul(out_sb[:, si, :], psums[si], mask_sb[:, si, :])
        nc.sync.dma_start(
            aslice.rearrange("(j p) t -> p j t", p=P), out_sb
        )


def _trace_mlp_kernel(tc, out, attn, xt):
    """out[6144,1024] = gelu(xT^T @ attn) for this core's row shard."""
    nc = tc.nc
    gelu = mybir.ActivationFunctionType.Gelu
    attn_r = attn.rearrange("(k p) t -> p k t", p=P)    # [128, 8, 1024]
    xt_r = xt.rearrange("(k p) r -> p k r", p=P)        # [128, 8, 6144]

    with ExitStack() as ctx:
        consts = ctx.enter_context(tc.tile_pool(name="consts", bufs=1))
        attn_sb = consts.tile([P, S_TILES, OUT_F], F16)
        for k in range(K_CHUNKS):
            nc.sync.dma_start(attn_sb[:, k, :], attn_r[:, k, :])

        xpool = ctx.enter_context(tc.tile_pool(name="xpool", bufs=8))
        opool = ctx.enter_context(tc.tile_pool(name="opool", bufs=3))
        mpool = ctx.enter_context(tc.tile_pool(name="main_psum", bufs=6, space="PSUM"))
        for rt in range(RT):
            xs = xpool.tile([P, K_CHUNKS, P], F16, name="xs")
            nc.sync.dma_start(xs, xt_r[:, :, rt * P:(rt + 1) * P])
            pa = mpool.tile([P, NB], F32, name="mp")
            pb = mpool.tile([P, NB], F32, name="mp")
            for k in range(K_CHUNKS):
                nc.tensor.matmul(
                    pa,
                    xs[:, k, :],
                    attn_sb[:, k, 0:NB],
                    start=(k == 0),
                    stop=(k == K_CHUNKS - 1),
                )
            for k in range(K_CHUNKS):
                nc.tensor.matmul(
                    pb,
                    xs[:, k, :],
                    attn_sb[:, k, NB:OUT_F],
                    start=(k == 0),
                    stop=(k == K_CHUNKS - 1),
                )
            ot = opool.tile([P, OUT_F], F32, name="ot")
            nc.scalar.activation(ot[:, 0:NB], pa, gelu)
            nc.scalar.activation(ot[:, NB:OUT_F], pb, gelu)
            nc.sync.dma_start(out[rt * P:(rt + 1) * P, :], ot)


_NC_CACHE = {}
LAST_RESULTS = None


def _build_attn():
    if "attn" in _NC_CACHE:
        return _NC_CACHE["attn"]
    nc = bacc.Bacc("TRN2", target_bir_lowering=False, debug=False,
                   num_devices=N_CORES)
    fc1s = nc.dram_tensor("fc1s", [KH, S_SL], F16, kind="ExternalInput").ap()
    fc2ts = nc.dram_tensor("fc2ts", [KH, T_SL], F16, kind="ExternalInput").ap()
    masks = nc.dram_tensor("masks", [S_SL, T_SL], F32, kind="ExternalInput").ap()
    aslice = nc.dram_tensor("aslice", [S_SL, T_SL], F16, kind="ExternalOutput").ap()
    with tile.TileContext(nc) as tc:
        _trace_attn_kernel(tc, aslice, fc1s, fc2ts, masks)
    nc.compile()
    _NC_CACHE["attn"] = nc
    return nc


def _build_mlp():
    if "mlp" in _NC_CACHE:
        return _NC_CACHE["mlp"]
    nc = bacc.Bacc("TRN2", target_bir_lowering=False, debug=False,
                   num_devices=N_CORES)
    attn = nc.dram_tensor("attn", [IN_F, OUT_F], F16, kind="ExternalInput").ap()
    xt = nc.dram_tensor("xt", [IN_F, ROWS_PC], F16, kind="ExternalInput").ap()
    out = nc.dram_tensor("out", [ROWS_PC, OUT_F], F32, kind="ExternalOutput").ap()
    with tile.TileContext(nc) as tc:
        _trace_mlp_kernel(tc, out, attn, xt)
    nc.compile()
    _NC_CACHE["mlp"] = nc
    return nc


def _run(nc, in_maps, **kwargs):
    return bass_utils.run_bass_kernel_spmd(
        nc, in_maps, core_ids=list(range(N_CORES)), **kwargs
    )


def kernel(x, fc1_w, fc2_w, fc2_b, sparse_mask, **run_kwargs):
    global LAST_RESULTS
    nc_a = _build_attn()
    nc_b = _build_mlp()

    # --- host prep: K-extended fp16 weight slices (layout only) ---
    fc1e = np.concatenate(
        [
            np.asarray(fc1_w, np.float32),
            np.ones((1, IN_F), np.float32),
            np.zeros((P - 1, IN_F), np.float32),
        ],
        axis=0,
    ).astype(np.float16)
    fc2te = np.concatenate(
        [
            np.asarray(fc2_w, np.float32).T,
            np.asarray(fc2_b, np.float32)[None, :],
            np.zeros((P - 1, OUT_F), np.float32),
        ],
        axis=0,
    ).astype(np.float16)
    mask = np.asarray(sparse_mask, np.float32)

    in_maps_a = []
    for c in range(N_CORES):
        si, tj = divmod(c, T_SH)
        in_maps_a.append({
            "fc1s": np.ascontiguousarray(fc1e[:, si * S_SL:(si + 1) * S_SL]),
            "fc2ts": np.ascontiguousarray(fc2te[:, tj * T_SL:(tj + 1) * T_SL]),
            "masks": np.ascontiguousarray(
                mask[si * S_SL:(si + 1) * S_SL, tj * T_SL:(tj + 1) * T_SL]
            ),
        })

    res_a = _run(nc_a, in_maps_a, **run_kwargs)

    # --- host gather of attn slices (pure concatenation) ---
    attn_full = np.empty((IN_F, OUT_F), np.float16)
    for c in range(N_CORES):
        si, tj = divmod(c, T_SH)
        attn_full[si * S_SL:(si + 1) * S_SL, tj * T_SL:(tj + 1) * T_SL] = (
            res_a.results[c]["aslice"]
        )

    x_flat = np.asarray(x, np.float32).reshape(ROWS, IN_F)
    in_maps_b = []
    for c in range(N_CORES):
        xt_c = np.ascontiguousarray(
            x_flat[c * ROWS_PC:(c + 1) * ROWS_PC].T.astype(np.float16)
        )
        in_maps_b.append({"attn": attn_full, "xt": xt_c})

    res_b = _run(nc_b, in_maps_b, **run_kwargs)
    LAST_RESULTS = (res_a, res_b)
    out = np.concatenate(
        [res_b.results[c]["out"] for c in range(N_CORES)], axis=0
    )
    return out.reshape(B, D, OUT_F)


# revision 15
# speedup vs baseline: 1.8271x; 1.1178x over previous
"""Trainium2 Bass kernel for RandomSparseNewMlp.

Math (reference):
    attn = (einsum('ds,td->st', fc1_w, fc2_w) + fc2_b) * sparse_mask   # [1024, 1024]
    out  = gelu_erf(einsum('bds,st->bdt', x, attn))                    # [64, 768, 1024]

Strategy (8 cores, SPMD, two NEFF dispatches, no collectives):
  NEFF A ("attn"): the [1024,1024] attn matrix is 2D-sharded over the 8
    cores (4-way along s, 2-way along t) — each core computes one
    [256, 512] slice from its fc1/fc2^T column slices, applies bias
    (folded into the contraction as an extra K-row: ones row in fc1,
    bias row in fc2^T, K padded 4096 -> 4224 = 33*128) and the sparse
    mask, and returns the masked fp16 slice.  The host concatenates the
    8 slices (pure layout, no arithmetic).
  NEFF B ("mlp"): data-parallel shard of x over batch; core c handles
    rows [c*6144, (c+1)*6144) of the flattened [49152, 1024] x, computes
    gelu(x @ attn) with the gathered attn as a replicated input.

  All matmul operands are fp16: full PE rate (1 cycle/row), 2-byte
  weight loads (LDWEIGHTS hides under the moving-operand stream), ~5e-4
  element precision, half the HBM traffic of fp32.  PSUM accumulation
  is fp32.  x is host-pre-transposed (xT layout [1024, rows]) so the
  contraction dim lands on SBUF partitions with clean contiguous DMA.
  GELU (erf-exact) is fused into the PSUM->SBUF eviction on ScalarE.
"""

import numpy as np
from contextlib import ExitStack

import concourse.bass as bass  # noqa: F401  (engine registration side effects)
import concourse.mybir as mybir
import concourse.tile as tile
from concourse import bacc
from concourse import bass_utils

P = 128
B, D = 64, 768
IN_F, HID_F, OUT_F = 1024, 4096, 1024
N_CORES = 8
ROWS = B * D                    # 49152
ROWS_PC = ROWS // N_CORES       # 6144
KH = HID_F + P                  # 4224 = 33*128 (hidden + bias/ones row, padded)
KD = KH // P                    # 33
S_TILES = IN_F // P             # 8
K_CHUNKS = IN_F // P            # 8
RT = ROWS_PC // P               # 48
NB = 512                        # matmul moving free dim / PSUM bank
S_SH, T_SH = 4, 2               # attn sharding grid: 4 along s, 2 along t
S_SL = IN_F // S_SH             # 256 rows of attn per core
T_SL = OUT_F // T_SH            # 512 cols of attn per core

F32 = mybir.dt.float32
F16 = mybir.dt.float16


def _trace_attn_kernel(tc, aslice, fc1s, fc2ts, masks):
    """Per-core attn slice: aslice[256,512] = (fc1s^T @ fc2ts) * masks.

    fc1s  [4224, 256]  fp16 : fc1 (K-extended) columns for this core's s-rows
    fc2ts [4224, 512]  fp16 : fc2^T (K-extended) columns for this core's t-cols
    masks [256, 512]   f32  : sparse-mask slice
    """
    nc = tc.nc
    fc1_r = fc1s.rearrange("(k p) s -> p k s", p=P)     # [128, 33, 256]
    fc2_r = fc2ts.rearrange("(k p) t -> p k t", p=P)    # [128, 33, 512]
    mask_r = masks.rearrange("(j p) t -> p j t", p=P)   # [128, 2, 512]

    with ExitStack() as ctx:
        spool = ctx.enter_context(tc.tile_pool(name="spool", bufs=1))
        ppool = ctx.enter_context(tc.tile_pool(name="ppool", bufs=2, space="PSUM"))
        # Whole weight slices resident in SBUF, loaded in ~512KB batched
        # DMAs (the per-strip version was DMA-issue-rate bound: ~70 small
        # DMAs x ~0.7us issue time serialized on the queue).
        f2_sb = spool.tile([P, KD, T_SL], F16)
        f1_sb = spool.tile([P, KD, S_SL], F16)
        mask_sb = spool.tile([P, 2, T_SL], F32)
        out_sb = spool.tile([P, 2, T_SL], F16)
        F2C, F1C = 4, 8   # kd-strips per DMA: 512 KB per transfer each
        f1_next = 0
        for c in range(0, KD, F2C):
            n = min(F2C, KD - c)
            nc.sync.dma_start(f2_sb[:, c:c + n, :], fc2_r[:, c:c + n, :])
            if (c // F2C) % 2 == 0 and f1_next < KD:
                m = min(F1C, KD - f1_next)
                nc.sync.dma_start(
                    f1_sb[:, f1_next:f1_next + m, :],
                    fc1_r[:, f1_next:f1_next + m, :],
                )
                f1_next += m
        while f1_next < KD:
            m = min(F1C, KD - f1_next)
            nc.sync.dma_start(
                f1_sb[:, f1_next:f1_next + m, :],
                fc1_r[:, f1_next:f1_next + m, :],
            )
            f1_next += m
        psums = [ppool.tile([P, NB], F32, name="ap") for _ in range(2)]
        for kd in range(KD):
            for si in range(2):
                nc.tensor.matmul(
                    psums[si],
                    f1_sb[:, kd, si * P:(si + 1) * P],
                    f2_sb[:, kd, :],
                    start=(kd == 0),
                    stop=(kd == KD - 1),
                )
        for j in range(2):
            nc.sync.dma_start(mask_sb[:, j, :], mask_r[:, j, :])
        for si in range(2):
            nc.vector.tensor_mul(out_sb[:, si, :], psums[si], mask_sb[:, si, :])
        nc.sync.dma_start(
            aslice.rearrange("(j p) t -> p j t", p=P), out_sb
        )


def _trace_mlp_kernel(tc, out, attn, xt):
    """out[6144,1024] = gelu(xT^T @ attn) for this core's row shard."""
    nc = tc.nc
    gelu = mybir.ActivationFunctionType.Gelu
    attn_r = attn.rearrange("(k p) t -> p k t", p=P)    # [128, 8, 1024]
    xt_r = xt.rearrange("(k p) r -> p k r", p=P)        # [128, 8, 6144]

    with ExitStack() as ctx:
        consts = ctx.enter_context(tc.tile_pool(name="consts", bufs=1))
        attn_sb = consts.tile([P, S_TILES, OUT_F], F16)
        xpool = ctx.enter_context(tc.tile_pool(name="xpool", bufs=8))
        opool = ctx.enter_context(tc.tile_pool(name="opool", bufs=3))
        mpool = ctx.enter_context(tc.tile_pool(name="main_psum", bufs=6, space="PSUM"))

        # First x strip ahead of the attn chunks so the first matmul's
        # inputs don't queue behind 2MB of attn transfers.
        xs0 = xpool.tile([P, K_CHUNKS, P], F16, name="xs")
        nc.sync.dma_start(xs0, xt_r[:, :, 0:P])
        for k in range(K_CHUNKS):
            nc.sync.dma_start(attn_sb[:, k, :], attn_r[:, k, :])

        for rt in range(RT):
            if rt == 0:
                xs = xs0
            else:
                xs = xpool.tile([P, K_CHUNKS, P], F16, name="xs")
                nc.sync.dma_start(xs, xt_r[:, :, rt * P:(rt + 1) * P])
            pa = mpool.tile([P, NB], F32, name="mp")
            pb = mpool.tile([P, NB], F32, name="mp")
            for k in range(K_CHUNKS):
                nc.tensor.matmul(
                    pa,
                    xs[:, k, :],
                    attn_sb[:, k, 0:NB],
                    start=(k == 0),
                    stop=(k == K_CHUNKS - 1),
                )
            for k in range(K_CHUNKS):
                nc.tensor.matmul(
                    pb,
                    xs[:, k, :],
                    attn_sb[:, k, NB:OUT_F],
                    start=(k == 0),
                    stop=(k == K_CHUNKS - 1),
                )
            ot = opool.tile([P, OUT_F], F32, name="ot")
            nc.scalar.activation(ot[:, 0:NB], pa, gelu)
            nc.scalar.activation(ot[:, NB:OUT_F], pb, gelu)
            nc.sync.dma_start(out[rt * P:(rt + 1) * P, :], ot)


_NC_CACHE = {}
LAST_RESULTS = None


def _build_attn():
    if "attn" in _NC_CACHE:
        return _NC_CACHE["attn"]
    nc = bacc.Bacc("TRN2", target_bir_lowering=False, debug=False,
                   num_devices=N_CORES)
    fc1s = nc.dram_tensor("fc1s", [KH, S_SL], F16, kind="ExternalInput").ap()
    fc2ts = nc.dram_tensor("fc2ts", [KH, T_SL], F16, kind="ExternalInput").ap()
    masks = nc.dram_tensor("masks", [S_SL, T_SL], F32, kind="ExternalInput").ap()
    aslice = nc.dram_tensor("aslice", [S_SL, T_SL], F16, kind="ExternalOutput").ap()
    with tile.TileContext(nc) as tc:
        _trace_attn_kernel(tc, aslice, fc1s, fc2ts, masks)
    nc.compile()
    _NC_CACHE["attn"] = nc
    return nc


def _build_mlp():
    if "mlp" in _NC_CACHE:
        return _NC_CACHE["mlp"]
    nc = bacc.Bacc("TRN2", target_bir_lowering=False, debug=False,
                   num_devices=N_CORES)
    attn = nc.dram_tensor("attn", [IN_F, OUT_F], F16, kind="ExternalInput").ap()
    xt = nc.dram_tensor("xt", [IN_F, ROWS_PC], F16, kind="ExternalInput").ap()
    out = nc.dram_tensor("out", [ROWS_PC, OUT_F], F32, kind="ExternalOutput").ap()
    with tile.TileContext(nc) as tc:
        _trace_mlp_kernel(tc, out, attn, xt)
    nc.compile()
    _NC_CACHE["mlp"] = nc
    return nc


def _run(nc, in_maps, **kwargs):
    return bass_utils.run_bass_kernel_spmd(
        nc, in_maps, core_ids=list(range(N_CORES)), **kwargs
    )


def kernel(x, fc1_w, fc2_w, fc2_b, sparse_mask, **run_kwargs):
    global LAST_RESULTS
    nc_a = _build_attn()
    nc_b = _build_mlp()

    # --- host prep: K-extended fp16 weight slices (layout only) ---
    fc1e = np.concatenate(
        [
            np.asarray(fc1_w, np.float32),
            np.ones((1, IN_F), np.float32),
            np.zeros((P - 1, IN_F), np.float32),
        ],
        axis=0,
    ).astype(np.float16)
    fc2te = np.concatenate(
        [
            np.asarray(fc2_w, np.float32).T,
            np.asarray(fc2_b, np.float32)[None, :],
            np.zeros((P - 1, OUT_F), np.float32),
        ],
        axis=0,
    ).astype(np.float16)
    mask = np.asarray(sparse_mask, np.float32)

    in_maps_a = []
    for c in range(N_CORES):
        si, tj = divmod(c, T_SH)
        in_maps_a.append({
            "fc1s": np.ascontiguousarray(fc1e[:, si * S_SL:(si + 1) * S_SL]),
            "fc2ts": np.ascontiguousarray(fc2te[:, tj * T_SL:(tj + 1) * T_SL]),
            "masks": np.ascontiguousarray(
                mask[si * S_SL:(si + 1) * S_SL, tj * T_SL:(tj + 1) * T_SL]
            ),
        })

    res_a = _run(nc_a, in_maps_a, **run_kwargs)

    # --- host gather of attn slices (pure concatenation) ---
    attn_full = np.empty((IN_F, OUT_F), np.float16)
    for c in range(N_CORES):
        si, tj = divmod(c, T_SH)
        attn_full[si * S_SL:(si + 1) * S_SL, tj * T_SL:(tj + 1) * T_SL] = (
            res_a.results[c]["aslice"]
        )

    x_flat = np.asarray(x, np.float32).reshape(ROWS, IN_F)
    in_maps_b = []
    for c in range(N_CORES):
        xt_c = np.ascontiguousarray(
            x_flat[c * ROWS_PC:(c + 1) * ROWS_PC].T.astype(np.float16)
        )
        in_maps_b.append({"attn": attn_full, "xt": xt_c})

    res_b = _run(nc_b, in_maps_b, **run_kwargs)
    LAST_RESULTS = (res_a, res_b)
    out = np.concatenate(
        [res_b.results[c]["out"] for c in range(N_CORES)], axis=0
    )
    return out.reshape(B, D, OUT_F)


# revision 17
# speedup vs baseline: 1.8337x; 1.0036x over previous
"""Trainium2 Bass kernel for RandomSparseNewMlp.

Math (reference):
    attn = (einsum('ds,td->st', fc1_w, fc2_w) + fc2_b) * sparse_mask   # [1024, 1024]
    out  = gelu_erf(einsum('bds,st->bdt', x, attn))                    # [64, 768, 1024]

Strategy (8 cores, SPMD, two NEFF dispatches, no collectives):
  NEFF A ("attn"): the [1024,1024] attn matrix is 2D-sharded over the 8
    cores (4-way along s, 2-way along t) — each core computes one
    [256, 512] slice from its fc1/fc2^T column slices, applies bias
    (folded into the contraction as an extra K-row: ones row in fc1,
    bias row in fc2^T, K padded 4096 -> 4224 = 33*128) and the sparse
    mask, and returns the masked fp16 slice.  The host concatenates the
    8 slices (pure layout, no arithmetic).
  NEFF B ("mlp"): data-parallel shard of x over batch; core c handles
    rows [c*6144, (c+1)*6144) of the flattened [49152, 1024] x, computes
    gelu(x @ attn) with the gathered attn as a replicated input.

  All matmul operands are fp16: full PE rate (1 cycle/row), 2-byte
  weight loads (LDWEIGHTS hides under the moving-operand stream), ~5e-4
  element precision, half the HBM traffic of fp32.  PSUM accumulation
  is fp32.  x is host-pre-transposed (xT layout [1024, rows]) so the
  contraction dim lands on SBUF partitions with clean contiguous DMA.
  GELU (erf-exact) is fused into the PSUM->SBUF eviction on ScalarE.
"""

import numpy as np
from contextlib import ExitStack

import concourse.bass as bass  # noqa: F401  (engine registration side effects)
import concourse.mybir as mybir
import concourse.tile as tile
from concourse import bacc
from concourse import bass_utils

P = 128
B, D = 64, 768
IN_F, HID_F, OUT_F = 1024, 4096, 1024
N_CORES = 8
ROWS = B * D                    # 49152
ROWS_PC = ROWS // N_CORES       # 6144
KH = HID_F + P                  # 4224 = 33*128 (hidden + bias/ones row, padded)
KD = KH // P                    # 33
S_TILES = IN_F // P             # 8
K_CHUNKS = IN_F // P            # 8
RT = ROWS_PC // P               # 48
NB = 512                        # matmul moving free dim / PSUM bank
S_SH, T_SH = 4, 2               # attn sharding grid: 4 along s, 2 along t
S_SL = IN_F // S_SH             # 256 rows of attn per core
T_SL = OUT_F // T_SH            # 512 cols of attn per core

F32 = mybir.dt.float32
F16 = mybir.dt.float16


def _trace_attn_kernel(tc, aslice, fc1s, fc2ts, masks):
    """Per-core attn slice: aslice[256,512] = (fc1s^T @ fc2ts) * masks.

    fc1s  [4224, 256]  fp16 : fc1 (K-extended) columns for this core's s-rows
    fc2ts [4224, 512]  fp16 : fc2^T (K-extended) columns for this core's t-cols
    masks [256, 512]   f32  : sparse-mask slice
    """
    nc = tc.nc
    fc1_r = fc1s.rearrange("(k p) s -> p k s", p=P)     # [128, 33, 256]
    fc2_r = fc2ts.rearrange("(k p) t -> p k t", p=P)    # [128, 33, 512]
    mask_r = masks.rearrange("(j p) t -> p j t", p=P)   # [128, 2, 512]

    with ExitStack() as ctx:
        spool = ctx.enter_context(tc.tile_pool(name="spool", bufs=1))
        ppool = ctx.enter_context(tc.tile_pool(name="ppool", bufs=2, space="PSUM"))
        # Whole weight slices resident in SBUF, loaded in ~512KB batched
        # DMAs (the per-strip version was DMA-issue-rate bound: ~70 small
        # DMAs x ~0.7us issue time serialized on the queue).
        f2_sb = spool.tile([P, KD, T_SL], F16)
        f1_sb = spool.tile([P, KD, S_SL], F16)
        mask_sb = spool.tile([P, 2, T_SL], F32)
        out_sb = spool.tile([P, 2, T_SL], F16)
        F2C, F1C = 4, 8   # kd-strips per DMA: 512 KB per transfer each
        f1_next = 0
        for c in range(0, KD, F2C):
            n = min(F2C, KD - c)
            nc.sync.dma_start(f2_sb[:, c:c + n, :], fc2_r[:, c:c + n, :])
            if (c // F2C) % 2 == 0 and f1_next < KD:
                m = min(F1C, KD - f1_next)
                nc.sync.dma_start(
                    f1_sb[:, f1_next:f1_next + m, :],
                    fc1_r[:, f1_next:f1_next + m, :],
                )
                f1_next += m
        while f1_next < KD:
            m = min(F1C, KD - f1_next)
            nc.sync.dma_start(
                f1_sb[:, f1_next:f1_next + m, :],
                fc1_r[:, f1_next:f1_next + m, :],
            )
            f1_next += m
        psums = [ppool.tile([P, NB], F32, name="ap") for _ in range(2)]
        for kd in range(KD):
            for si in range(2):
                nc.tensor.matmul(
                    psums[si],
                    f1_sb[:, kd, si * P:(si + 1) * P],
                    f2_sb[:, kd, :],
                    start=(kd == 0),
                    stop=(kd == KD - 1),
                )
        for j in range(2):
            nc.sync.dma_start(mask_sb[:, j, :], mask_r[:, j, :])
        for si in range(2):
            nc.vector.tensor_mul(out_sb[:, si, :], psums[si], mask_sb[:, si, :])
        nc.sync.dma_start(
            aslice.rearrange("(j p) t -> p j t", p=P), out_sb
        )


def _trace_mlp_kernel(tc, out, attn, xt):
    """out[6144,1024] = gelu(xT^T @ attn) for this core's row shard."""
    nc = tc.nc
    gelu = mybir.ActivationFunctionType.Gelu
    attn_r = attn.rearrange("(k p) t -> p k t", p=P)    # [128, 8, 1024]
    xt_r = xt.rearrange("(k p) r -> p k r", p=P)        # [128, 8, 6144]

    with ExitStack() as ctx:
        consts = ctx.enter_context(tc.tile_pool(name="consts", bufs=1))
        attn_sb = consts.tile([P, S_TILES, OUT_F], F16)
        xpool = ctx.enter_context(tc.tile_pool(name="xpool", bufs=8))
        opool = ctx.enter_context(tc.tile_pool(name="opool", bufs=3))
        mpool = ctx.enter_context(tc.tile_pool(name="main_psum", bufs=8, space="PSUM"))

        # First x strip ahead of the attn chunks so the first matmul's
        # inputs don't queue behind 2MB of attn transfers.
        xs0 = xpool.tile([P, K_CHUNKS, P], F16, name="xs")
        nc.sync.dma_start(xs0, xt_r[:, :, 0:P])
        nc.sync.dma_start(attn_sb[:, 0:1, :], attn_r[:, 0:1, :])
        nc.sync.dma_start(attn_sb[:, 1:4, :], attn_r[:, 1:4, :])
        nc.sync.dma_start(attn_sb[:, 4:8, :], attn_r[:, 4:8, :])

        for rt in range(RT):
            if rt == 0:
                xs = xs0
            else:
                xs = xpool.tile([P, K_CHUNKS, P], F16, name="xs")
                nc.sync.dma_start(xs, xt_r[:, :, rt * P:(rt + 1) * P])
            pa = mpool.tile([P, NB], F32, name="mp")
            pb = mpool.tile([P, NB], F32, name="mp")
            for k in range(K_CHUNKS):
                nc.tensor.matmul(
                    pa,
                    xs[:, k, :],
                    attn_sb[:, k, 0:NB],
                    start=(k == 0),
                    stop=(k == K_CHUNKS - 1),
                )
            for k in range(K_CHUNKS):
                nc.tensor.matmul(
                    pb,
                    xs[:, k, :],
                    attn_sb[:, k, NB:OUT_F],
                    start=(k == 0),
                    stop=(k == K_CHUNKS - 1),
                )
            ot = opool.tile([P, OUT_F], F32, name="ot")
            nc.scalar.activation(ot[:, 0:NB], pa, gelu)
            nc.scalar.activation(ot[:, NB:OUT_F], pb, gelu)
            nc.sync.dma_start(out[rt * P:(rt + 1) * P, :], ot)


_NC_CACHE = {}
LAST_RESULTS = None


def _build_attn():
    if "attn" in _NC_CACHE:
        return _NC_CACHE["attn"]
    nc = bacc.Bacc("TRN2", target_bir_lowering=False, debug=False,
                   num_devices=N_CORES)
    fc1s = nc.dram_tensor("fc1s", [KH, S_SL], F16, kind="ExternalInput").ap()
    fc2ts = nc.dram_tensor("fc2ts", [KH, T_SL], F16, kind="ExternalInput").ap()
    masks = nc.dram_tensor("masks", [S_SL, T_SL], F32, kind="ExternalInput").ap()
    aslice = nc.dram_tensor("aslice", [S_SL, T_SL], F16, kind="ExternalOutput").ap()
    with tile.TileContext(nc) as tc:
        _trace_attn_kernel(tc, aslice, fc1s, fc2ts, masks)
    nc.compile()
    _NC_CACHE["attn"] = nc
    return nc


def _build_mlp():
    if "mlp" in _NC_CACHE:
        return _NC_CACHE["mlp"]
    nc = bacc.Bacc("TRN2", target_bir_lowering=False, debug=False,
                   num_devices=N_CORES)
    attn = nc.dram_tensor("attn", [IN_F, OUT_F], F16, kind="ExternalInput").ap()
    xt = nc.dram_tensor("xt", [IN_F, ROWS_PC], F16, kind="ExternalInput").ap()
    out = nc.dram_tensor("out", [ROWS_PC, OUT_F], F32, kind="ExternalOutput").ap()
    with tile.TileContext(nc) as tc:
        _trace_mlp_kernel(tc, out, attn, xt)
    nc.compile()
    _NC_CACHE["mlp"] = nc
    return nc


def _run(nc, in_maps, **kwargs):
    return bass_utils.run_bass_kernel_spmd(
        nc, in_maps, core_ids=list(range(N_CORES)), **kwargs
    )


def kernel(x, fc1_w, fc2_w, fc2_b, sparse_mask, **run_kwargs):
    global LAST_RESULTS
    nc_a = _build_attn()
    nc_b = _build_mlp()

    # --- host prep: K-extended fp16 weight slices (layout only) ---
    fc1e = np.concatenate(
        [
            np.asarray(fc1_w, np.float32),
            np.ones((1, IN_F), np.float32),
            np.zeros((P - 1, IN_F), np.float32),
        ],
        axis=0,
    ).astype(np.float16)
    fc2te = np.concatenate(
        [
            np.asarray(fc2_w, np.float32).T,
            np.asarray(fc2_b, np.float32)[None, :],
            np.zeros((P - 1, OUT_F), np.float32),
        ],
        axis=0,
    ).astype(np.float16)
    mask = np.asarray(sparse_mask, np.float32)

    in_maps_a = []
    for c in range(N_CORES):
        si, tj = divmod(c, T_SH)
        in_maps_a.append({
            "fc1s": np.ascontiguousarray(fc1e[:, si * S_SL:(si + 1) * S_SL]),
            "fc2ts": np.ascontiguousarray(fc2te[:, tj * T_SL:(tj + 1) * T_SL]),
            "masks": np.ascontiguousarray(
                mask[si * S_SL:(si + 1) * S_SL, tj * T_SL:(tj + 1) * T_SL]
            ),
        })

    res_a = _run(nc_a, in_maps_a, **run_kwargs)

    # --- host gather of attn slices (pure concatenation) ---
    attn_full = np.empty((IN_F, OUT_F), np.float16)
    for c in range(N_CORES):
        si, tj = divmod(c, T_SH)
        attn_full[si * S_SL:(si + 1) * S_SL, tj * T_SL:(tj + 1) * T_SL] = (
            res_a.results[c]["aslice"]
        )

    x_flat = np.asarray(x, np.float32).reshape(ROWS, IN_F)
    in_maps_b = []
    for c in range(N_CORES):
        xt_c = np.ascontiguousarray(
            x_flat[c * ROWS_PC:(c + 1) * ROWS_PC].T.astype(np.float16)
        )
        in_maps_b.append({"attn": attn_full, "xt": xt_c})

    res_b = _run(nc_b, in_maps_b, **run_kwargs)
    LAST_RESULTS = (res_a, res_b)
    out = np.concatenate(
        [res_b.results[c]["out"] for c in range(N_CORES)], axis=0
    )
    return out.reshape(B, D, OUT_F)
